# revision 16
# baseline (speedup 1.0000x reference)
"""DIEN forward-loss kernel for Trainium2, SPMD over 8 NeuronCores.

Sharding: data-parallel over batch (B=256 -> 32 rows/core), embedding table
replicated. Each core runs: embedding gather + max_norm renorm -> GRU (interest
extraction) -> aux BCE partial -> AUGRU (interest evolution, attention == 1)
-> AllGather(h, item, aux_sum) -> replicated final MLP with Dice batchnorm
(full-batch stats) + BCE -> identical scalar loss on every core.

All recurrence state is kept transposed ([D on partitions, batch on free]) so
the per-step matmuls need no transposes. x-side matmuls are chunked (8 steps,
N=256) and accumulated in PSUM; per-step h-side matmuls accumulate into the
same PSUM chunk so the sigmoid reads a single buffer.
"""
import numpy as np
import concourse.bass as bass
import concourse.bacc as bacc
import concourse.mybir as mybir
import concourse.tile as tile
from concourse.bass_utils import run_bass_kernel_spmd
from concourse.masks import make_identity

F32 = mybir.dt.float32
BF16 = mybir.dt.bfloat16
I32 = mybir.dt.int32
AF = mybir.ActivationFunctionType
OP = mybir.AluOpType

# problem constants (hardcoded; kernel.py must be self-contained)
B, L, D, NV = 256, 200, 128, 500000
NCORES = 8
BL = B // NCORES          # 32 batch rows per core
NT = L * BL               # 6400 (t,b) pairs per core
NTIL = NT // 128          # 50 gather tiles
CH = 8                    # recurrence chunk (timesteps per x-side matmul)
CW = CH * BL              # 256 columns per chunk
NCHUNK = L // CH          # 25
LAG = CH                  # AUGRU lags GRU by one chunk
EPS_BN = 1e-5
DICE_A = 0.1
ALPHA = 0.2
MAGIC = 0x5F3759DF


def _rsqrt(nc, pool, v, out, shape, iters=4):
    """out = 1/sqrt(v) elementwise on DVE only (no ACT tables).

    Quake seed via int bit-trick, then Newton iterations
    y <- y * (1.5 - 0.5 * v * y^2). v must be >= 0; v == 0 gives a large
    finite value (callers clamp with min()).
    """
    p, n = shape
    iv = out.bitcast(I32)
    nc.vector.tensor_scalar(
        out=iv, in0=v.bitcast(I32), scalar1=1, scalar2=None,
        op0=OP.arith_shift_right,
    )
    # magic - i  ==  (i xor -1) + (magic + 1)
    nc.vector.tensor_scalar(
        out=iv, in0=iv, scalar1=-1, scalar2=None,
        op0=OP.bitwise_xor,
    )
    nc.vector.tensor_scalar(
        out=iv, in0=iv, scalar1=MAGIC + 1, scalar2=None,
        op0=OP.add,
    )
    t = pool.tile([p, n], F32, tag="rsqrt_t")
    for _ in range(iters):
        nc.vector.tensor_tensor(out=t[:], in0=v, in1=out, op=OP.mult)
        nc.vector.tensor_tensor(out=t[:], in0=t[:], in1=out, op=OP.mult)
        nc.vector.tensor_scalar(
            out=t[:], in0=t[:], scalar1=-0.5, scalar2=1.5,
            op0=OP.mult, op1=OP.add,
        )
        nc.vector.tensor_tensor(out=out, in0=out, in1=t[:], op=OP.mult)


def build_bass(upto="full"):
    # upto: "A" (gather only), "G" (+GRU), "GA" (+AUGRU), "X" (+aux/gather),
    #       "full". Truncated builds write a debug value to out.
    nc = bacc.Bacc("TRN2", target_bir_lowering=False, num_devices=NCORES)

    # ---------------- kernel parameters ----------------
    emb = nc.declare_dram_parameter("emb", [NV, D], F32, isOutput=False)
    idx_h = nc.declare_dram_parameter("idx_h", [128, NTIL], I32, isOutput=False)
    y_h = nc.declare_dram_parameter("y_h", [128, NTIL], F32, isOutput=False)
    idx_t = nc.declare_dram_parameter("idx_t", [BL, 1], I32, isOutput=False)
    wihT = nc.declare_dram_parameter("wihT", [D, 3 * D], BF16, isOutput=False)
    whhT = nc.declare_dram_parameter("whhT", [D, 3 * D], BF16, isOutput=False)
    bias_gi = nc.declare_dram_parameter("bias_gi", [1, 2 * D], F32, isOutput=False)
    bihn = nc.declare_dram_parameter("bihn", [1, D], F32, isOutput=False)
    bhhn = nc.declare_dram_parameter("bhhn", [1, D], F32, isOutput=False)
    Wall = nc.declare_dram_parameter("Wall", [D, 3 * D], BF16, isOutput=False)
    Uall = nc.declare_dram_parameter("Uall", [D, 3 * D], BF16, isOutput=False)
    bias_ur = nc.declare_dram_parameter("bias_ur", [1, 2 * D], F32, isOutput=False)
    bh_aug = nc.declare_dram_parameter("bh_aug", [1, D], F32, isOutput=False)
    W1 = nc.declare_dram_parameter("W1", [2 * D, D], F32, isOutput=False)
    b1 = nc.declare_dram_parameter("b1", [1, D], F32, isOutput=False)
    W2 = nc.declare_dram_parameter("W2", [D, D // 2], F32, isOutput=False)
    b2 = nc.declare_dram_parameter("b2", [1, D // 2], F32, isOutput=False)
    Wf = nc.declare_dram_parameter("Wf", [D // 2, 1], F32, isOutput=False)
    bf = nc.declare_dram_parameter("bf", [1, 1], F32, isOutput=False)
    h0T = nc.declare_dram_parameter("h0T", [D, BL], BF16, isOutput=False)
    y_t = nc.declare_dram_parameter("y_t", [1, B], F32, isOutput=False)
    out_p = nc.declare_dram_parameter("out", [1, 1], F32, isOutput=True)

    # internal DRAM for the collective
    ploc = nc.dram_tensor("ploc", [BL + 1, 2 * D + 1], F32)
    gall = nc.dram_tensor("gall", [NCORES * (BL + 1), 2 * D + 1], F32)

    with tile.TileContext(nc) as tc:
        with (
            tc.tile_pool(name="persist", bufs=1) as pp,
            tc.tile_pool(name="work", bufs=2) as wk,
            tc.tile_pool(name="ps_ck", bufs=3, space="PSUM") as pck,
            tc.tile_pool(name="ps_st", bufs=2, space="PSUM") as pst,
            tc.tile_pool(name="ps_g", bufs=1, space="PSUM") as psg,
        ):
            # ---------------- constants / weights to SBUF ----------------
            ident = pp.tile([128, 128], F32, tag="ident")
            make_identity(nc, ident[:])
            ident_bf = pp.tile([128, 128], BF16, tag="ident_bf")
            nc.vector.tensor_copy(ident_bf[:], ident[:])
            ones_col = pp.tile([128, 1], F32, tag="ones_col")
            nc.gpsimd.memset(ones_col[:], 1.0)
            zeros_b = pp.tile([128, BL], BF16, tag="zeros_b")
            nc.gpsimd.memset(zeros_b[:], 0.0)

            def load(name_ap, shape, tag, dt=F32):
                t = pp.tile(shape, dt, tag=tag)
                nc.sync.dma_start(out=t[:], in_=name_ap[:])
                return t

            wihT_s = load(wihT, [D, 3 * D], "wihT", BF16)
            whhT_s = load(whhT, [D, 3 * D], "whhT", BF16)
            Wall_s = load(Wall, [D, 3 * D], "Wall", BF16)
            Uall_s = load(Uall, [D, 3 * D], "Uall", BF16)
            W1a_s = pp.tile([D, D], F32, tag="W1a")
            nc.sync.dma_start(out=W1a_s[:], in_=W1[0:D, :])
            W1b_s = pp.tile([D, D], F32, tag="W1b")
            nc.sync.dma_start(out=W1b_s[:], in_=W1[D:2 * D, :])
            b1_s = load(b1, [1, D], "b1")
            W2_s = load(W2, [D, D // 2], "W2")
            b2_s = load(b2, [1, D // 2], "b2")
            Wf_s = load(Wf, [D // 2, 1], "Wf")
            bf_s = load(bf, [1, 1], "bf")
            y_t_s = load(y_t, [1, B], "y_t")
            y_h_s = load(y_h, [128, NTIL], "y_h")
            hA = load(h0T, [D, BL], "hA", BF16)  # AUGRU state (in-place updated)

            idx_s = pp.tile([128, NTIL], I32, tag="idx_s")
            nc.sync.dma_start(out=idx_s[:], in_=idx_h[:])
            idx_t_s = pp.tile([BL, 1], I32, tag="idx_t_s")
            nc.sync.dma_start(out=idx_t_s[:], in_=idx_t[:])

            # bias APs for per-partition ACT bias: need [128, 1] views.
            # bias_* are [1, N] in SBUF -> we need them per-partition instead.
            # Load transposed copies via DMA from DRAM with AP rearrange.
            def load_col(src_ap, tag):
                t = pp.tile([D, 1], F32, tag=tag)
                nc.sync.dma_start(out=t[:], in_=src_ap.rearrange("o d -> d o"))
                return t

            bihn_c = load_col(bihn[:], "bihn_c")
            bhhn_c = load_col(bhhn[:], "bhhn_c")
            bh_aug_c = load_col(bh_aug[:], "bh_aug_c")
            br_c = load_col(bias_gi[0:1, 0:D], "br_c")
            bz_c = load_col(bias_gi[0:1, D:2 * D], "bz_c")
            bu_c = load_col(bias_ur[0:1, 0:D], "bu_c")
            bur_c = load_col(bias_ur[0:1, D:2 * D], "bur_c")

            # persistent big buffers
            ET = pp.tile([128, NT], BF16, tag="ET")         # e^T  [d, (t,b)]
            outsT = pp.tile([128, NT], BF16, tag="outsT")   # GRU outs^T
            s_all = pp.tile([128, NTIL], F32, tag="s_all")  # aux logits
            ss_all = pp.tile([128, NTIL], F32, tag="ss_all")
            erows = pp.tile([128, NT], F32, tag="erows")    # gathered rows

            # =========== Phase A: embedding gather + renorm + transpose ====
            for k in range(NTIL):
                sl = erows[:, 128 * k:128 * (k + 1)]
                nc.gpsimd.indirect_dma_start(
                    out=sl, out_offset=None, in_=emb[:],
                    in_offset=bass.IndirectOffsetOnAxis(ap=idx_s[:, k:k + 1], axis=0),
                )
                sq = wk.tile([128, 128], F32, tag="sq_scr")
                nc.scalar.activation(sq[:], sl, AF.Square,
                                     accum_out=ss_all[:, k:k + 1])
            scale = pp.tile([128, NTIL], F32, tag="scale")
            _rsqrt(nc, wk, ss_all[:], scale[:], [128, NTIL])
            nc.vector.tensor_scalar_min(out=scale[:], in0=scale[:], scalar1=1.0)
            for k in range(NTIL):
                sl = erows[:, 128 * k:128 * (k + 1)]
                nc.vector.tensor_scalar(
                    out=sl, in0=sl, scalar1=scale[:, k:k + 1], scalar2=None,
                    op0=OP.mult,
                )
                tp = psg.tile([128, 128], F32, tag="gram")
                nc.tensor.transpose(out=tp[:], in_=sl, identity=ident[:])
                nc.vector.tensor_copy(ET[:, 128 * k:128 * (k + 1)], tp[:])

            # target item: gather 32 rows + renorm (keep row layout)
            itemr = pp.tile([BL, D], F32, tag="itemr")
            nc.gpsimd.indirect_dma_start(
                out=itemr[:], out_offset=None, in_=emb[:],
                in_offset=bass.IndirectOffsetOnAxis(ap=idx_t_s[:, :1], axis=0),
            )
            sqt = wk.tile([BL, D], F32, tag="sqt")
            sst = wk.tile([BL, 1], F32, tag="sst")
            nc.scalar.activation(sqt[:], itemr[:], AF.Square, accum_out=sst[:])
            sct = wk.tile([BL, 1], F32, tag="sct")
            _rsqrt(nc, wk, sst[:], sct[:], [BL, 1])
            nc.vector.tensor_scalar_min(out=sct[:], in0=sct[:], scalar1=1.0)
            nc.vector.tensor_scalar(
                out=itemr[:], in0=itemr[:], scalar1=sct[:], scalar2=None,
                op0=OP.mult,
            )

            if upto == "A":
                dbg = wk.tile([1, 1], F32, tag="res")
                nc.vector.reduce_sum(out=dbg[:], in_=ET[0:1, 0:128],
                                     axis=mybir.AxisListType.X)
                nc.sync.dma_start(out=out_p[:], in_=dbg[:])

            # =========== Phase B/C/D: GRU + aux grams + AUGRU ==============
            gruRZ = None
            gruN = None
            augUR = None
            augH = None
            for slot in range(L + LAG if upto != "A" else 0):
                tg = slot
                ta = slot - LAG
                # ---- GRU x-side chunk: 3 matmuls + biased copies to SBUF ----
                if tg < L and tg % CH == 0:
                    ecols = ET[:, tg * BL: tg * BL + CW]
                    rz_sb = wk.tile([128, 2 * CW], F32, tag="g_rzck")
                    ckR = pck.tile([128, CW], F32, tag="ck")
                    nc.tensor.matmul(ckR[:], wihT_s[:, 0:D], ecols,
                                     start=True, stop=True)
                    nc.vector.tensor_scalar(
                        out=rz_sb[:, 0:CW], in0=ckR[:],
                        scalar1=br_c[:, 0:1], scalar2=None, op0=OP.add)
                    ckZ = pck.tile([128, CW], F32, tag="ck")
                    nc.tensor.matmul(ckZ[:], wihT_s[:, D:2 * D], ecols,
                                     start=True, stop=True)
                    nc.vector.tensor_scalar(
                        out=rz_sb[:, CW:2 * CW], in0=ckZ[:],
                        scalar1=bz_c[:, 0:1], scalar2=None, op0=OP.add)
                    ckN = pck.tile([128, CW], F32, tag="ck")
                    nc.tensor.matmul(ckN[:], wihT_s[:, 2 * D:3 * D], ecols,
                                     start=True, stop=True)
                    gin_sb = wk.tile([128, CW], F32, tag="g_nck")
                    nc.scalar.activation(gin_sb[:], ckN[:], AF.Copy)
                # ---- GRU step ----
                if tg < L:
                    o = tg % CH
                    h_prev = (zeros_b[:, 0:BL] if tg == 0
                              else outsT[:, (tg - 1) * BL: tg * BL])
                    hz = pst.tile([128, 3 * BL], F32, tag="ghz")
                    nc.tensor.matmul(hz[:, 2 * BL:3 * BL], whhT_s[:, 2 * D:3 * D],
                                     h_prev, start=True, stop=False)
                    nc.tensor.matmul(hz[:, 0:BL], whhT_s[:, 0:D], h_prev,
                                     start=False, stop=False)
                    nc.tensor.matmul(hz[:, BL:2 * BL], whhT_s[:, D:2 * D], h_prev,
                                     start=False, stop=True)
                    srz = wk.tile([128, 2 * BL], F32, tag="g_srz")
                    nc.vector.tensor_tensor(
                        out=srz[:, 0:BL], in0=hz[:, 0:BL],
                        in1=rz_sb[:, o * BL:(o + 1) * BL], op=OP.add)
                    nc.vector.tensor_tensor(
                        out=srz[:, BL:2 * BL], in0=hz[:, BL:2 * BL],
                        in1=rz_sb[:, CW + o * BL:CW + (o + 1) * BL], op=OP.add)
                    rz = wk.tile([128, 2 * BL], F32, tag="g_rz")
                    nc.scalar.activation(rz[:], srz[:], AF.Sigmoid)
                    t1 = wk.tile([128, BL], F32, tag="g_t1")
                    nc.vector.tensor_scalar(
                        out=t1[:], in0=hz[:, 2 * BL:3 * BL],
                        scalar1=bhhn_c[:, 0:1], scalar2=None, op0=OP.add)
                    nc.vector.tensor_tensor(out=t1[:], in0=t1[:],
                                            in1=rz[:, 0:BL], op=OP.mult)
                    t2 = wk.tile([128, BL], F32, tag="g_t2")
                    nc.vector.tensor_tensor(
                        out=t2[:], in0=t1[:],
                        in1=gin_sb[:, o * BL:(o + 1) * BL], op=OP.add)
                    nt = wk.tile([128, BL], F32, tag="g_n")
                    nc.scalar.activation(nt[:], t2[:], AF.Tanh,
                                         bias=bihn_c[:, 0:1])
                    d1 = wk.tile([128, BL], F32, tag="g_d1")
                    nc.vector.tensor_tensor(out=d1[:], in0=h_prev, in1=nt[:],
                                            op=OP.subtract)
                    d2 = wk.tile([128, BL], F32, tag="g_d2")
                    nc.vector.tensor_tensor(out=d2[:], in0=rz[:, BL:2 * BL],
                                            in1=d1[:], op=OP.mult)
                    nc.vector.tensor_tensor(
                        out=outsT[:, tg * BL:(tg + 1) * BL], in0=nt[:],
                        in1=d2[:], op=OP.add)
                # ---- aux gram: diag(outs_blk^T @ E_blk) ----
                if tg < L and tg % 4 == 3:
                    blk = tg // 4
                    gps = psg.tile([128, 128], F32, tag="gram")
                    nc.tensor.matmul(
                        gps[:], outsT[:, 128 * blk:128 * (blk + 1)],
                        ET[:, 128 * blk:128 * (blk + 1)], start=True, stop=True)
                    gsc = wk.tile([128, 128], F32, tag="gram_scr")
                    nc.vector.tensor_tensor(out=gsc[:], in0=gps[:],
                                            in1=ident[:], op=OP.mult)
                    nc.vector.reduce_sum(out=s_all[:, blk:blk + 1], in_=gsc[:],
                                         axis=mybir.AxisListType.X)
                if upto == "G":
                    continue
                # ---- AUGRU x-side chunk ----
                if 0 <= ta < L and ta % CH == 0:
                    ocols = outsT[:, ta * BL: ta * BL + CW]
                    ur_sb = wk.tile([128, 2 * CW], F32, tag="a_urck")
                    ckU = pck.tile([128, CW], F32, tag="ck")
                    nc.tensor.matmul(ckU[:], Wall_s[:, 0:D], ocols,
                                     start=True, stop=True)
                    nc.vector.tensor_scalar(
                        out=ur_sb[:, 0:CW], in0=ckU[:],
                        scalar1=bu_c[:, 0:1], scalar2=None, op0=OP.add)
                    ckR2 = pck.tile([128, CW], F32, tag="ck")
                    nc.tensor.matmul(ckR2[:], Wall_s[:, D:2 * D], ocols,
                                     start=True, stop=True)
                    nc.vector.tensor_scalar(
                        out=ur_sb[:, CW:2 * CW], in0=ckR2[:],
                        scalar1=bur_c[:, 0:1], scalar2=None, op0=OP.add)
                    ckH = pck.tile([128, CW], F32, tag="ck")
                    nc.tensor.matmul(ckH[:], Wall_s[:, 2 * D:3 * D], ocols,
                                     start=True, stop=True)
                    xh_sb = wk.tile([128, CW], F32, tag="a_hck")
                    nc.scalar.activation(xh_sb[:], ckH[:], AF.Copy)
                # ---- AUGRU step (attention weight == 1) ----
                if 0 <= ta < L:
                    o2 = ta % CH
                    hz2 = pst.tile([128, 3 * BL], F32, tag="ahz")
                    nc.tensor.matmul(hz2[:, 2 * BL:3 * BL], Uall_s[:, 2 * D:3 * D],
                                     hA[:], start=True, stop=False)
                    nc.tensor.matmul(hz2[:, 0:BL], Uall_s[:, 0:D], hA[:],
                                     start=False, stop=False)
                    nc.tensor.matmul(hz2[:, BL:2 * BL], Uall_s[:, D:2 * D], hA[:],
                                     start=False, stop=True)
                    sur = wk.tile([128, 2 * BL], F32, tag="a_sur")
                    nc.vector.tensor_tensor(
                        out=sur[:, 0:BL], in0=hz2[:, 0:BL],
                        in1=ur_sb[:, o2 * BL:(o2 + 1) * BL], op=OP.add)
                    nc.vector.tensor_tensor(
                        out=sur[:, BL:2 * BL], in0=hz2[:, BL:2 * BL],
                        in1=ur_sb[:, CW + o2 * BL:CW + (o2 + 1) * BL], op=OP.add)
                    ur = wk.tile([128, 2 * BL], F32, tag="a_ur")
                    nc.scalar.activation(ur[:], sur[:], AF.Sigmoid)
                    t1a = wk.tile([128, BL], F32, tag="a_t1")
                    nc.vector.tensor_tensor(out=t1a[:], in0=ur[:, BL:2 * BL],
                                            in1=hz2[:, 2 * BL:3 * BL], op=OP.mult)
                    t2a = wk.tile([128, BL], F32, tag="a_t2")
                    nc.vector.tensor_tensor(
                        out=t2a[:], in0=t1a[:],
                        in1=xh_sb[:, o2 * BL:(o2 + 1) * BL], op=OP.add)
                    hht = wk.tile([128, BL], F32, tag="a_hh")
                    nc.scalar.activation(hht[:], t2a[:], AF.Tanh,
                                         bias=bh_aug_c[:, 0:1])
                    d1a = wk.tile([128, BL], F32, tag="a_d1")
                    nc.vector.tensor_tensor(out=d1a[:], in0=hht[:], in1=hA[:],
                                            op=OP.subtract)
                    d2a = wk.tile([128, BL], F32, tag="a_d2")
                    nc.vector.tensor_tensor(out=d2a[:], in0=ur[:, 0:BL],
                                            in1=d1a[:], op=OP.mult)
                    nc.vector.tensor_tensor(out=hA[:], in0=hA[:], in1=d2a[:],
                                            op=OP.add)

            if upto == "G":
                dbg = wk.tile([1, 1], F32, tag="res")
                nc.vector.reduce_sum(out=dbg[:], in_=outsT[0:1, NT - 128:NT],
                                     axis=mybir.AxisListType.X)
                nc.sync.dma_start(out=out_p[:], in_=dbg[:])
            if upto == "GA":
                dbg = wk.tile([1, 1], F32, tag="res")
                nc.vector.reduce_sum(out=dbg[:], in_=hA[0:1, :],
                                     axis=mybir.AxisListType.X)
                nc.sync.dma_start(out=out_p[:], in_=dbg[:])
            do_aux = upto in ("X", "full")

            if do_aux:
                # =========== aux BCE partial sum (Exp/Ln table) ================
                ebuf = pp.tile([128, NTIL], F32, tag="ebuf")
                nc.scalar.activation(ebuf[:], s_all[:], AF.Exp)
                nc.vector.tensor_scalar_add(out=ebuf[:], in0=ebuf[:], scalar1=1.0)
                sp = pp.tile([128, NTIL], F32, tag="sp")
                nc.scalar.activation(sp[:], ebuf[:], AF.Ln)
                spm = pp.tile([128, NTIL], F32, tag="spm")
                nc.vector.tensor_tensor(out=spm[:], in0=sp[:], in1=s_all[:],
                                        op=OP.subtract)
                nc.vector.tensor_scalar_min(out=spm[:], in0=spm[:], scalar1=100.0)
                nc.vector.tensor_scalar_min(out=sp[:], in0=sp[:], scalar1=100.0)
                # loss_i = sp + y*(spm - sp)
                nc.vector.tensor_tensor(out=spm[:], in0=spm[:], in1=sp[:],
                                        op=OP.subtract)
                nc.vector.tensor_tensor(out=spm[:], in0=y_h_s[:], in1=spm[:],
                                        op=OP.mult)
                nc.vector.tensor_tensor(out=sp[:], in0=sp[:], in1=spm[:], op=OP.add)
                rsum = wk.tile([128, 1], F32, tag="rsum")
                nc.vector.reduce_sum(out=rsum[:], in_=sp[:],
                                     axis=mybir.AxisListType.X)
                aux_ps = psg.tile([1, 1], F32, tag="gram")
                nc.tensor.matmul(aux_ps[:], rsum[:], ones_col[:, 0:1],
                                 start=True, stop=True)
                aux_sc = wk.tile([1, 1], F32, tag="aux_sc")
                nc.vector.tensor_copy(aux_sc[:], aux_ps[:])

                # =========== pack + AllGather ==================================
                # hA -> rows [BL, D]
                hAf = wk.tile([D, BL], F32, tag="hAf")
                nc.vector.tensor_copy(hAf[:], hA[:])
                hrow_ps = psg.tile([BL, D], F32, tag="gram")
                nc.tensor.transpose(out=hrow_ps[:], in_=hAf[:], identity=ident[:])
                stage = pp.tile([BL + 1, 2 * D + 1], F32, tag="stage")
                nc.gpsimd.memset(stage[:], 0.0)
                nc.vector.tensor_copy(stage[0:BL, 0:D], hrow_ps[:])
                nc.vector.tensor_copy(stage[0:BL, D:2 * D], itemr[:])
                nc.vector.tensor_copy(stage[BL:BL + 1, 0:1], aux_sc[:])
                nc.sync.dma_start(out=ploc[:], in_=stage[:])
                nc.gpsimd.collective_compute(
                    "AllGather", OP.bypass,
                    replica_groups=[list(range(NCORES))],
                    ins=[ploc[:]], outs=[gall[:]],
                )

            if upto == "X":
                dbg = wk.tile([1, 1], F32, tag="res")
                nc.vector.tensor_copy(dbg[:], aux_sc[:])
                nc.sync.dma_start(out=out_p[:], in_=dbg[:])

            if upto == "full":
                # =========== replicated final MLP ==============================
                PW = 2 * D + 1  # gall row width
                hT_all = pp.tile([D, B], F32, tag="hT_all")
                iT_all = pp.tile([D, B], F32, tag="iT_all")
                for half in range(2):
                    hr = wk.tile([128, 128], F32, tag="hr_half")
                    ir_ = wk.tile([128, 128], F32, tag="ir_half")
                    for j in range(4):
                        c = 4 * half + j
                        r0 = (BL + 1) * c
                        nc.sync.dma_start(out=hr[BL * j:BL * (j + 1), :],
                                          in_=gall[r0:r0 + BL, 0:D])
                        nc.sync.dma_start(out=ir_[BL * j:BL * (j + 1), :],
                                          in_=gall[r0:r0 + BL, D:2 * D])
                    tp = psg.tile([128, 128], F32, tag="gram")
                    nc.tensor.transpose(out=tp[:], in_=hr[:], identity=ident[:])
                    nc.vector.tensor_copy(hT_all[:, 128 * half:128 * (half + 1)],
                                          tp[:])
                    tp2 = psg.tile([128, 128], F32, tag="gram")
                    nc.tensor.transpose(out=tp2[:], in_=ir_[:], identity=ident[:])
                    nc.vector.tensor_copy(iT_all[:, 128 * half:128 * (half + 1)],
                                          tp2[:])
                aux8 = wk.tile([1, NCORES], F32, tag="aux8")
                for c in range(NCORES):
                    nc.sync.dma_start(out=aux8[0:1, c:c + 1],
                                      in_=gall[(BL + 1) * c + BL:(BL + 1) * c + BL + 1,
                                               0:1])
                aux_tot = wk.tile([1, 1], F32, tag="aux_tot")
                nc.vector.reduce_sum(out=aux_tot[:], in_=aux8[:],
                                     axis=mybir.AxisListType.X)

                ones_b = pp.tile([1, B], F32, tag="ones_b")
                nc.gpsimd.memset(ones_b[:], 1.0)

                def dice(z_ps, pdim):
                    """Dice on z (psum [pdim, B], batch on free axis).
                    Returns SBUF tile z*(0.1+0.9*sigmoid((z-mu)/std))."""
                    m = wk.tile([pdim, 1], F32, tag="dice_m")
                    nc.vector.reduce_sum(out=m[:], in_=z_ps[:],
                                         axis=mybir.AxisListType.X)
                    nc.vector.tensor_scalar_mul(out=m[:], in0=m[:], scalar1=1.0 / B)
                    xc = wk.tile([pdim, B], F32, tag="dice_xc")
                    nc.vector.tensor_scalar(out=xc[:], in0=z_ps[:], scalar1=m[:],
                                            scalar2=None, op0=OP.subtract)
                    sq2 = wk.tile([pdim, B], F32, tag="dice_sq")
                    vs = wk.tile([pdim, 1], F32, tag="dice_vs")
                    nc.scalar.activation(sq2[:], xc[:], AF.Square, accum_out=vs[:])
                    nc.vector.tensor_scalar(out=vs[:], in0=vs[:], scalar1=1.0 / B,
                                            scalar2=EPS_BN, op0=OP.mult, op1=OP.add)
                    inv = wk.tile([pdim, 1], F32, tag="dice_inv")
                    _rsqrt(nc, wk, vs[:], inv[:], [pdim, 1])
                    pr = wk.tile([pdim, B], F32, tag="dice_p")
                    nc.scalar.activation(pr[:], xc[:], AF.Sigmoid, scale=inv[:, 0:1])
                    nc.vector.tensor_scalar(out=pr[:], in0=pr[:], scalar1=1 - DICE_A,
                                            scalar2=DICE_A, op0=OP.mult, op1=OP.add)
                    zd = wk.tile([pdim, B], F32, tag="dice_zd")
                    nc.vector.tensor_tensor(out=zd[:], in0=z_ps[:], in1=pr[:],
                                            op=OP.mult)
                    return zd

                z1_ps = pck.tile([128, B], F32, tag="ck")
                nc.tensor.matmul(z1_ps[:], W1a_s[:], hT_all[:],
                                 start=True, stop=False)
                nc.tensor.matmul(z1_ps[:], W1b_s[:], iT_all[:],
                                 start=False, stop=False)
                nc.tensor.matmul(z1_ps[:], b1_s[0:1, :], ones_b[0:1, :],
                                 start=False, stop=True)
                z1d = dice(z1_ps, 128)

                z2_ps = pck.tile([D // 2, B], F32, tag="ck")
                nc.tensor.matmul(z2_ps[:], W2_s[:, :], z1d[:],
                                 start=True, stop=False)
                nc.tensor.matmul(z2_ps[:], b2_s[0:1, :], ones_b[0:1, :],
                                 start=False, stop=True)
                z2d = dice(z2_ps, D // 2)

                s_ps = pck.tile([1, B], F32, tag="ck")
                nc.tensor.matmul(s_ps[:], Wf_s[:, 0:1], z2d[:],
                                 start=True, stop=False)
                nc.tensor.matmul(s_ps[:], bf_s[0:1, 0:1], ones_b[0:1, :],
                                 start=False, stop=True)
                s_sb = wk.tile([1, B], F32, tag="s_sb")
                nc.vector.tensor_copy(s_sb[:], s_ps[:])

                # rec BCE over the full batch (replicated on every core)
                e2 = wk.tile([1, B], F32, tag="e2")
                nc.scalar.activation(e2[:], s_sb[:], AF.Exp)
                nc.vector.tensor_scalar_add(out=e2[:], in0=e2[:], scalar1=1.0)
                sp2 = wk.tile([1, B], F32, tag="sp2")
                nc.scalar.activation(sp2[:], e2[:], AF.Ln)
                spm2 = wk.tile([1, B], F32, tag="spm2")
                nc.vector.tensor_tensor(out=spm2[:], in0=sp2[:], in1=s_sb[:],
                                        op=OP.subtract)
                nc.vector.tensor_scalar_min(out=spm2[:], in0=spm2[:], scalar1=100.0)
                nc.vector.tensor_scalar_min(out=sp2[:], in0=sp2[:], scalar1=100.0)
                nc.vector.tensor_tensor(out=spm2[:], in0=spm2[:], in1=sp2[:],
                                        op=OP.subtract)
                nc.vector.tensor_tensor(out=spm2[:], in0=y_t_s[:], in1=spm2[:],
                                        op=OP.mult)
                nc.vector.tensor_tensor(out=sp2[:], in0=sp2[:], in1=spm2[:],
                                        op=OP.add)
                rec_sum = wk.tile([1, 1], F32, tag="rec_sum")
                nc.vector.reduce_sum(out=rec_sum[:], in_=sp2[:],
                                     axis=mybir.AxisListType.X)

                nc.vector.tensor_scalar_mul(out=aux_tot[:], in0=aux_tot[:],
                                            scalar1=ALPHA / (B * L))
                nc.vector.tensor_scalar_mul(out=rec_sum[:], in0=rec_sum[:],
                                            scalar1=1.0 / B)
                res = wk.tile([1, 1], F32, tag="res")
                nc.vector.tensor_tensor(out=res[:], in0=aux_tot[:], in1=rec_sum[:],
                                        op=OP.add)
                nc.sync.dma_start(out=out_p[:], in_=res[:])
    nc.compile()
    return nc


_NC_CACHE = None


def _get_nc():
    global _NC_CACHE
    if _NC_CACHE is None:
        import os
        _NC_CACHE = build_bass(os.environ.get("KERNEL_UPTO", "full"))
    return _NC_CACHE


def _prep_inputs(inputs):
    """Build the 8 per-core input maps from the full problem inputs."""
    import ml_dtypes
    bf16 = ml_dtypes.bfloat16
    f32 = np.float32
    emb = np.ascontiguousarray(inputs["emb"], dtype=f32)
    seqs = np.asarray(inputs["history_seqs"])          # [B, L] int32
    labs = np.asarray(inputs["history_labels"])        # [B, L, 1] int32
    tgt = np.asarray(inputs["target_item"])            # [B] int32
    tl = np.asarray(inputs["target_label"]).astype(f32)  # [B]

    w_ih = np.asarray(inputs["w_ih"], dtype=f32)
    w_hh = np.asarray(inputs["w_hh"], dtype=f32)
    b_ih = np.asarray(inputs["b_ih"], dtype=f32)
    b_hh = np.asarray(inputs["b_hh"], dtype=f32)
    wihT = np.ascontiguousarray(w_ih.T).astype(bf16)
    whhT = np.ascontiguousarray(w_hh.T).astype(bf16)
    bias_gi = (b_ih[:2 * D] + b_hh[:2 * D]).reshape(1, 2 * D)
    bihn = b_ih[2 * D:].reshape(1, D)
    bhhn = b_hh[2 * D:].reshape(1, D)

    Wall = np.ascontiguousarray(
        np.concatenate([inputs["Wu"], inputs["Wr"], inputs["Wh"]], axis=1),
        dtype=f32).astype(bf16)
    Uall = np.ascontiguousarray(
        np.concatenate([inputs["Uu"], inputs["Ur"], inputs["Uh"]], axis=1),
        dtype=f32).astype(bf16)
    bias_ur = np.concatenate(
        [np.asarray(inputs["bu"], dtype=f32).reshape(-1),
         np.asarray(inputs["br"], dtype=f32).reshape(-1)]).reshape(1, 2 * D)
    bh_aug = np.asarray(inputs["bh"], dtype=f32).reshape(1, D)

    W1 = np.ascontiguousarray(inputs["W1"], dtype=f32)
    b1 = np.asarray(inputs["b1"], dtype=f32).reshape(1, D)
    W2 = np.ascontiguousarray(inputs["W2"], dtype=f32)
    b2 = np.asarray(inputs["b2"], dtype=f32).reshape(1, D // 2)
    Wf = np.ascontiguousarray(inputs["Wf"], dtype=f32)
    bf = np.asarray(inputs["bf"], dtype=f32).reshape(1, 1)
    h0 = np.asarray(inputs["h0"], dtype=f32)
    y_t_full = tl.reshape(1, B)

    shared = dict(emb=emb, wihT=wihT, whhT=whhT, bias_gi=bias_gi, bihn=bihn,
                  bhhn=bhhn, Wall=Wall, Uall=Uall, bias_ur=bias_ur,
                  bh_aug=bh_aug, W1=W1, b1=b1, W2=W2, b2=b2, Wf=Wf, bf=bf,
                  y_t=y_t_full)
    in_maps = []
    for c in range(NCORES):
        sl = slice(c * BL, (c + 1) * BL)
        # t-major flattening: n = t*BL + b  -> [128, NTIL] with n = 128k + p
        idx_f = np.ascontiguousarray(seqs[sl].T).reshape(-1)      # [NT]
        idx_h = np.ascontiguousarray(
            idx_f.reshape(NTIL, 128).T).astype(np.int32)
        y_f = np.ascontiguousarray(labs[sl, :, 0].T).reshape(-1).astype(f32)
        y_h = np.ascontiguousarray(y_f.reshape(NTIL, 128).T)
        h0T = np.ascontiguousarray(h0[sl].T).astype(bf16)
        idx_tc = tgt[sl].reshape(BL, 1).astype(np.int32)
        m = dict(shared)
        m.update(idx_h=idx_h, y_h=y_h, idx_t=idx_tc, h0T=h0T)
        in_maps.append(m)
    return in_maps


def kernel(**inputs) -> np.ndarray:
    nc = _get_nc()
    in_maps = _prep_inputs(inputs)
    res = run_bass_kernel_spmd(nc, in_maps, core_ids=list(range(NCORES)))
    out = np.asarray(res.results[0]["out"], dtype=np.float32)
    return out.reshape(())



# revision 17
# speedup vs baseline: 1.2730x; 1.2730x over previous
"""DIEN forward-loss kernel for Trainium2, SPMD over 8 NeuronCores. v2.

v1 -> v2:
- PE-linearity: W@h_t = W@q_t + W@zh_t (two bf16 movings), so the next step's
  matmuls consume q (post-tanh DVE product) directly and the state
  materialization h=q+zh happens off the critical chain
- phase A (gather + renorm + transpose) interleaved into the loop emission;
  only tiles 0..13 processed up front
- aux/rec BCE: sigmoid pairs written into one tile, single Ln per pair, and a
  bypass-dependency forces the aux table switch after the last AUGRU step
- tail: strided batched DMAs for the gathered h blocks, 2-iter dice rsqrt
"""
import numpy as np
import concourse.bass as bass
import concourse.bacc as bacc
import concourse.mybir as mybir
import concourse.tile as tile
from concourse.bass_utils import run_bass_kernel_spmd
from concourse.masks import make_identity

F32 = mybir.dt.float32
BF16 = mybir.dt.bfloat16
I32 = mybir.dt.int32
AF = mybir.ActivationFunctionType
OP = mybir.AluOpType

B, L, D, NV = 256, 200, 128, 500000
NCORES = 8
BL = B // NCORES          # 32
NT = L * BL               # 6400
NTIL = NT // 128          # 50
CH = 5
CW = CH * BL              # 160
LAG = CH
EPS_BN = 1e-5
DICE_A = 0.1
ALPHA = 0.2
MAGIC = 0x5F3759DF
D2 = D // 2
PRE_TILES = 14            # tiles renormed before the loop


def _rsqrt(nc, pool, v, out, shape, iters=3):
    p, n = shape
    iv = out.bitcast(I32)
    nc.vector.tensor_scalar(out=iv, in0=v.bitcast(I32), scalar1=1, scalar2=None,
                            op0=OP.arith_shift_right)
    nc.vector.tensor_scalar(out=iv, in0=iv, scalar1=-1, scalar2=None,
                            op0=OP.bitwise_xor)
    nc.vector.tensor_scalar(out=iv, in0=iv, scalar1=MAGIC + 1, scalar2=None,
                            op0=OP.add)
    t = pool.tile([p, n], F32, tag="rsqrt_t")
    for _ in range(iters):
        nc.vector.tensor_tensor(out=t[:, 0:n], in0=v, in1=out, op=OP.mult)
        nc.vector.tensor_tensor(out=t[:, 0:n], in0=t[:, 0:n], in1=out, op=OP.mult)
        nc.vector.tensor_scalar(out=t[:, 0:n], in0=t[:, 0:n], scalar1=-0.5,
                                scalar2=1.5, op0=OP.mult, op1=OP.add)
        nc.vector.tensor_tensor(out=out, in0=out, in1=t[:, 0:n], op=OP.mult)


def build_bass():
    nc = bacc.Bacc("TRN2", target_bir_lowering=False, num_devices=NCORES)

    emb = nc.declare_dram_parameter("emb", [NV, D], F32, isOutput=False)
    idx_h = nc.declare_dram_parameter("idx_h", [128, NTIL], I32, isOutput=False)
    y_h = nc.declare_dram_parameter("y_h", [128, NTIL], F32, isOutput=False)
    idx_t = nc.declare_dram_parameter("idx_t", [128, 2], I32, isOutput=False)
    wihT = nc.declare_dram_parameter("wihT", [D, 3 * D], BF16, isOutput=False)
    whhT = nc.declare_dram_parameter("whhT", [D, 3 * D], BF16, isOutput=False)
    b_rz = nc.declare_dram_parameter("b_rz", [1, 2 * D], BF16, isOutput=False)
    b_n = nc.declare_dram_parameter("b_n", [1, D], BF16, isOutput=False)
    bhhn = nc.declare_dram_parameter("bhhn", [1, D], F32, isOutput=False)
    Wall = nc.declare_dram_parameter("Wall", [D, 3 * D], BF16, isOutput=False)
    Uall = nc.declare_dram_parameter("Uall", [D, 3 * D], BF16, isOutput=False)
    b_urh = nc.declare_dram_parameter("b_urh", [1, 3 * D], BF16, isOutput=False)
    W1 = nc.declare_dram_parameter("W1", [2 * D, D], BF16, isOutput=False)
    b1 = nc.declare_dram_parameter("b1", [1, D], BF16, isOutput=False)
    W2 = nc.declare_dram_parameter("W2", [D, D2], BF16, isOutput=False)
    b2 = nc.declare_dram_parameter("b2", [1, D2], BF16, isOutput=False)
    Wf = nc.declare_dram_parameter("Wf", [D2, 1], BF16, isOutput=False)
    bf_ = nc.declare_dram_parameter("bf", [1, 1], BF16, isOutput=False)
    h0T = nc.declare_dram_parameter("h0T", [D, BL], BF16, isOutput=False)
    y_t = nc.declare_dram_parameter("y_t", [1, B], F32, isOutput=False)
    out_p = nc.declare_dram_parameter("out", [1, 1], F32, isOutput=True)

    ploc = nc.dram_tensor("ploc", [D, BL + 1], BF16)
    gall = nc.dram_tensor("gall", [NCORES * D, BL + 1], BF16)

    with tile.TileContext(nc) as tc:
        with (
            tc.tile_pool(name="persist", bufs=1) as pp,
            tc.tile_pool(name="work", bufs=2) as wk,
            tc.tile_pool(name="ps_gx", bufs=2, space="PSUM") as pgx,
            tc.tile_pool(name="ps_ax", bufs=2, space="PSUM") as pax,
            tc.tile_pool(name="ps_g", bufs=1, space="PSUM") as psg,
            tc.tile_pool(name="ps_mlp", bufs=1, space="PSUM") as pmlp,
            tc.tile_pool(name="ps_tp", bufs=2, space="PSUM") as ptp,
        ):
            ident = pp.tile([128, 128], F32, tag="ident")
            make_identity(nc, ident[:])
            ident_bf = pp.tile([128, 128], BF16, tag="ident_bf")
            nc.vector.tensor_copy(ident_bf[:], ident[:])
            ones160 = pp.tile([1, CW], BF16, tag="ones160")
            nc.gpsimd.memset(ones160[:], 1.0)
            ones_b = pp.tile([1, B], BF16, tag="ones_b")
            nc.gpsimd.memset(ones_b[:], 1.0)
            ones_col = pp.tile([128, 1], F32, tag="ones_col")
            nc.gpsimd.memset(ones_col[:], 1.0)
            zeros_b = pp.tile([128, BL], BF16, tag="zeros_b")
            nc.gpsimd.memset(zeros_b[:], 0.0)

            def load(src, shape, tag, dt=F32):
                t = pp.tile(shape, dt, tag=tag)
                nc.sync.dma_start(out=t[:], in_=src[:])
                return t

            wihT_s = load(wihT, [D, 3 * D], "wihT", BF16)
            whhT_s = load(whhT, [D, 3 * D], "whhT", BF16)
            Wall_s = load(Wall, [D, 3 * D], "Wall", BF16)
            Uall_s = load(Uall, [D, 3 * D], "Uall", BF16)
            b_rz_s = load(b_rz, [1, 2 * D], "b_rz", BF16)
            b_n_s = load(b_n, [1, D], "b_n", BF16)
            b_urh_s = load(b_urh, [1, 3 * D], "b_urh", BF16)
            W1a_s = pp.tile([D, D], BF16, tag="W1a")
            nc.sync.dma_start(out=W1a_s[:], in_=W1[0:D, :])
            W1b_s = pp.tile([D, D], BF16, tag="W1b")
            nc.sync.dma_start(out=W1b_s[:], in_=W1[D:2 * D, :])
            b1_s = load(b1, [1, D], "b1", BF16)
            W2_s = load(W2, [D, D2], "W2", BF16)
            b2_s = load(b2, [1, D2], "b2", BF16)
            Wf_s = load(Wf, [D2, 1], "Wf", BF16)
            bf_s = load(bf_, [1, 1], "bf", BF16)
            y_t_s = load(y_t, [1, B], "y_t")
            y_h_s = load(y_h, [128, NTIL], "y_h")
            hA = load(h0T, [D, BL], "hA", BF16)

            idx_s = pp.tile([128, NTIL], I32, tag="idx_s")
            nc.sync.dma_start(out=idx_s[:], in_=idx_h[:])
            idx_t_s = pp.tile([128, 2], I32, tag="idx_t_s")
            nc.sync.dma_start(out=idx_t_s[:], in_=idx_t[:])

            bhhn_c = pp.tile([D, 1], F32, tag="bhhn_c")
            nc.sync.dma_start(out=bhhn_c[:], in_=bhhn[:].rearrange("o d -> d o"))

            ET = pp.tile([128, NT], BF16, tag="ET")
            outsT = pp.tile([128, NT], BF16, tag="outsT")
            erows = pp.tile([128, NT], F32, tag="erows")
            itr = pp.tile([128, 2 * 128], F32, tag="itr")
            itT = pp.tile([128, B], BF16, tag="itT")
            ss_all = pp.tile([128, NTIL], F32, tag="ss_all")
            scale = pp.tile([128, NTIL], F32, tag="scale")
            ss_t = pp.tile([128, 2], F32, tag="ss_t")
            sc_t = pp.tile([128, 2], F32, tag="sc_t")
            s_all = pp.tile([128, NTIL], F32, tag="s_all")

            # ---------- phase A helpers ----------
            def gather_tile(k):
                nc.gpsimd.indirect_dma_start(
                    out=erows[:, 128 * k:128 * (k + 1)], out_offset=None,
                    in_=emb[:],
                    in_offset=bass.IndirectOffsetOnAxis(ap=idx_s[:, k:k + 1], axis=0),
                )

            def sq_tile(k):
                sl = erows[:, 128 * k:128 * (k + 1)]
                junk = wk.tile([128, 128], F32, tag="sqj")
                nc.vector.scalar_tensor_tensor(
                    out=junk[:], in0=sl, scalar=1.0, in1=sl,
                    op0=OP.mult, op1=OP.mult, accum_out=ss_all[:, k:k + 1])

            def rsq_group(k0, k1):
                _rsqrt(nc, wk, ss_all[:, k0:k1], scale[:, k0:k1], [128, k1 - k0])
                nc.vector.tensor_scalar_min(out=scale[:, k0:k1],
                                            in0=scale[:, k0:k1], scalar1=1.0)

            def finish_tile(k):
                sl = erows[:, 128 * k:128 * (k + 1)]
                esc = wk.tile([128, 128], BF16, tag="esc")
                nc.scalar.activation(esc[:], sl, AF.Copy, scale=scale[:, k:k + 1])
                tp = ptp.tile([128, 128], BF16, tag="tpb")
                nc.tensor.transpose(out=tp[:], in_=esc[:], identity=ident_bf[:])
                nc.scalar.copy(out=ET[:, 128 * k:128 * (k + 1)], in_=tp[:])

            # ---------- phase A prologue: items + first tiles ----------
            for k in range(PRE_TILES):
                gather_tile(k)
            for j in range(2):
                nc.gpsimd.indirect_dma_start(
                    out=itr[:, 128 * j:128 * (j + 1)], out_offset=None,
                    in_=emb[:],
                    in_offset=bass.IndirectOffsetOnAxis(ap=idx_t_s[:, j:j + 1], axis=0),
                )
            for j in range(2):
                sl = itr[:, 128 * j:128 * (j + 1)]
                junk = wk.tile([128, 128], F32, tag="sqj")
                nc.vector.scalar_tensor_tensor(
                    out=junk[:], in0=sl, scalar=1.0, in1=sl,
                    op0=OP.mult, op1=OP.mult, accum_out=ss_t[:, j:j + 1])
            _rsqrt(nc, wk, ss_t[:], sc_t[:], [128, 2])
            nc.vector.tensor_scalar_min(out=sc_t[:], in0=sc_t[:], scalar1=1.0)
            for j in range(2):
                sl = itr[:, 128 * j:128 * (j + 1)]
                esc = wk.tile([128, 128], BF16, tag="esc")
                nc.scalar.activation(esc[:], sl, AF.Copy, scale=sc_t[:, j:j + 1])
                tp = ptp.tile([128, 128], BF16, tag="tpb")
                nc.tensor.transpose(out=tp[:], in_=esc[:], identity=ident_bf[:])
                nc.scalar.copy(out=itT[:, 128 * j:128 * (j + 1)], in_=tp[:])
            for k in range(PRE_TILES):
                sq_tile(k)
            rsq_group(0, PRE_TILES)
            for k in range(PRE_TILES):
                finish_tile(k)

            # phase-A schedule for the remaining tiles, keyed by slot
            slot_gather = {}
            slot_sq = {}
            slot_rsq = {}
            slot_fin = {}
            for k in range(PRE_TILES, 32):             # group B
                slot_gather.setdefault(2 * (k - PRE_TILES), []).append(k)
                slot_sq.setdefault(2 * (k - PRE_TILES) + 2, []).append(k)
            slot_rsq[38] = (PRE_TILES, 32)
            for i, k in enumerate(range(PRE_TILES, 32)):
                slot_fin.setdefault(39 + i, []).append(k)
            for k in range(32, NTIL):                  # group C
                slot_gather.setdefault(2 * (k - 32) + 36, []).append(k)
                slot_sq.setdefault(2 * (k - 32) + 38, []).append(k)
            slot_rsq[74] = (32, NTIL)
            for i, k in enumerate(range(32, NTIL)):
                slot_fin.setdefault(75 + i, []).append(k)

            # =========== main loop ===============
            gxt = None
            axt = None
            qG = zhG = None        # GRU movings for next step (bf16)
            pA = ouA = None        # AUGRU movings
            for slot in range(L + LAG):
                tg = slot
                ta = slot - LAG
                # ---- GRU ----
                if tg < L:
                    o = tg % CH
                    if o == 0:
                        gx = pgx.tile([128, 512], F32, tag="gx")
                        gxt = gx
                        ecols = ET[:, tg * BL: tg * BL + CW]
                        nc.tensor.matmul(gxt[:, 0:CW], wihT_s[:, 0:D], ecols,
                                         start=True, stop=False)
                        nc.tensor.matmul(gxt[:, 0:CW], b_rz_s[0:1, 0:D],
                                         ones160[0:1, :], start=False, stop=False)
                        nc.tensor.matmul(gxt[:, CW:2 * CW], wihT_s[:, D:2 * D],
                                         ecols, start=True, stop=False)
                        nc.tensor.matmul(gxt[:, CW:2 * CW], b_rz_s[0:1, D:2 * D],
                                         ones160[0:1, :], start=False, stop=False)
                        nc.tensor.matmul(gxt[:, 2 * CW:3 * CW], wihT_s[:, 2 * D:3 * D],
                                         ecols, start=True, stop=False)
                        nc.tensor.matmul(gxt[:, 2 * CW:3 * CW], b_n_s[0:1, :],
                                         ones160[0:1, :], start=False, stop=True)
                    rsl = gxt[:, o * BL:(o + 1) * BL]
                    zsl = gxt[:, CW + o * BL:CW + (o + 1) * BL]
                    nsl = gxt[:, 3 * CW:3 * CW + BL]
                    if tg == 0:
                        nc.tensor.matmul(rsl, whhT_s[:, 0:D], zeros_b[:, 0:BL],
                                         start=False, stop=True)
                        nc.tensor.matmul(zsl, whhT_s[:, D:2 * D], zeros_b[:, 0:BL],
                                         start=False, stop=True)
                        nc.tensor.matmul(nsl, whhT_s[:, 2 * D:3 * D],
                                         zeros_b[:, 0:BL], start=True, stop=True)
                    else:
                        nc.tensor.matmul(rsl, whhT_s[:, 0:D], zhG[:],
                                         start=False, stop=False)
                        nc.tensor.matmul(zsl, whhT_s[:, D:2 * D], zhG[:],
                                         start=False, stop=False)
                        nc.tensor.matmul(nsl, whhT_s[:, 2 * D:3 * D], zhG[:],
                                         start=True, stop=False)
                        nc.tensor.matmul(rsl, whhT_s[:, 0:D], qG[:],
                                         start=False, stop=True)
                        nc.tensor.matmul(zsl, whhT_s[:, D:2 * D], qG[:],
                                         start=False, stop=True)
                        nc.tensor.matmul(nsl, whhT_s[:, 2 * D:3 * D], qG[:],
                                         start=False, stop=True)
                    hp = (zeros_b[:, 0:BL] if tg == 0
                          else outsT[:, (tg - 1) * BL: tg * BL])
                    rzv = gxt[:, 0:2 * CW].rearrange(
                        "p (g s b) -> p g s b", g=2, s=CH)[:, :, o, :]
                    rz = wk.tile([128, 2 * BL], F32, tag="g_rz")
                    nc.scalar.activation(rz[:], rzv, AF.Sigmoid)
                    t1 = wk.tile([128, BL], F32, tag="g_t1")
                    nc.vector.scalar_tensor_tensor(
                        out=t1[:], in0=nsl, scalar=bhhn_c[:, 0:1], in1=rz[:, 0:BL],
                        op0=OP.add, op1=OP.mult)
                    t2 = wk.tile([128, BL], F32, tag="g_t2")
                    nc.vector.tensor_tensor(
                        out=t2[:], in0=t1[:],
                        in1=gxt[:, 2 * CW + o * BL:2 * CW + (o + 1) * BL],
                        op=OP.add)
                    nt = wk.tile([128, BL], F32, tag="g_n")
                    nc.scalar.activation(nt[:], t2[:], AF.Tanh)
                    omz = wk.tile([128, BL], F32, tag="g_omz")
                    nc.gpsimd.tensor_scalar(out=omz[:], in0=rz[:, BL:2 * BL],
                                            scalar1=-1.0, scalar2=1.0,
                                            op0=OP.mult, op1=OP.add)
                    zh_n = wk.tile([128, BL], BF16, tag="g_zh")
                    nc.gpsimd.tensor_tensor(out=zh_n[:], in0=rz[:, BL:2 * BL],
                                            in1=hp, op=OP.mult)
                    q_n = wk.tile([128, BL], BF16, tag="g_q")
                    nc.vector.tensor_tensor(out=q_n[:], in0=omz[:], in1=nt[:],
                                            op=OP.mult)
                    nc.vector.tensor_tensor(out=outsT[:, tg * BL:(tg + 1) * BL],
                                            in0=q_n[:], in1=zh_n[:], op=OP.add)
                    qG, zhG = q_n, zh_n
                # ---- aux gram (one slot after the block completes) ----
                if (tg % 4 == 0 and 4 <= tg < L) or slot == L:
                    blk = (tg // 4 - 1) if tg < L else NTIL - 1
                    gps = psg.tile([128, 128], F32, tag="gram")
                    nc.tensor.matmul(gps[:], outsT[:, 128 * blk:128 * (blk + 1)],
                                     ET[:, 128 * blk:128 * (blk + 1)],
                                     start=True, stop=True)
                    gj = wk.tile([128, 128], F32, tag="gramj")
                    nc.vector.scalar_tensor_tensor(
                        out=gj[:], in0=gps[:], scalar=1.0, in1=ident[:],
                        op0=OP.mult, op1=OP.mult,
                        accum_out=s_all[:, blk:blk + 1])
                # ---- AUGRU ----
                if 0 <= ta < L:
                    o2 = ta % CH
                    if o2 == 0:
                        ax = pax.tile([128, 512], F32, tag="ax")
                        axt = ax
                        ocols = outsT[:, ta * BL: ta * BL + CW]
                        nc.tensor.matmul(axt[:, 0:CW], Wall_s[:, 0:D], ocols,
                                         start=True, stop=False)
                        nc.tensor.matmul(axt[:, 0:CW], b_urh_s[0:1, 0:D],
                                         ones160[0:1, :], start=False, stop=False)
                        nc.tensor.matmul(axt[:, CW:2 * CW], Wall_s[:, D:2 * D],
                                         ocols, start=True, stop=False)
                        nc.tensor.matmul(axt[:, CW:2 * CW], b_urh_s[0:1, D:2 * D],
                                         ones160[0:1, :], start=False, stop=False)
                        nc.tensor.matmul(axt[:, 2 * CW:3 * CW], Wall_s[:, 2 * D:3 * D],
                                         ocols, start=True, stop=False)
                        nc.tensor.matmul(axt[:, 2 * CW:3 * CW],
                                         b_urh_s[0:1, 2 * D:3 * D],
                                         ones160[0:1, :], start=False, stop=True)
                    usl = axt[:, o2 * BL:(o2 + 1) * BL]
                    r2sl = axt[:, CW + o2 * BL:CW + (o2 + 1) * BL]
                    n2sl = axt[:, 3 * CW:3 * CW + BL]
                    if ta == 0:
                        nc.tensor.matmul(usl, Uall_s[:, 0:D], hA[:],
                                         start=False, stop=True)
                        nc.tensor.matmul(r2sl, Uall_s[:, D:2 * D], hA[:],
                                         start=False, stop=True)
                        nc.tensor.matmul(n2sl, Uall_s[:, 2 * D:3 * D], hA[:],
                                         start=True, stop=True)
                    else:
                        nc.tensor.matmul(usl, Uall_s[:, 0:D], ouA[:],
                                         start=False, stop=False)
                        nc.tensor.matmul(r2sl, Uall_s[:, D:2 * D], ouA[:],
                                         start=False, stop=False)
                        nc.tensor.matmul(n2sl, Uall_s[:, 2 * D:3 * D], ouA[:],
                                         start=True, stop=False)
                        nc.tensor.matmul(usl, Uall_s[:, 0:D], pA[:],
                                         start=False, stop=True)
                        nc.tensor.matmul(r2sl, Uall_s[:, D:2 * D], pA[:],
                                         start=False, stop=True)
                        nc.tensor.matmul(n2sl, Uall_s[:, 2 * D:3 * D], pA[:],
                                         start=False, stop=True)
                    urv = axt[:, 0:2 * CW].rearrange(
                        "p (g s b) -> p g s b", g=2, s=CH)[:, :, o2, :]
                    ur = wk.tile([128, 2 * BL], F32, tag="a_ur")
                    nc.scalar.activation(ur[:], urv, AF.Sigmoid)
                    t1a = wk.tile([128, BL], F32, tag="a_t1")
                    nc.vector.tensor_tensor(out=t1a[:], in0=ur[:, BL:2 * BL],
                                            in1=n2sl, op=OP.mult)
                    t2a = wk.tile([128, BL], F32, tag="a_t2")
                    nc.vector.tensor_tensor(
                        out=t2a[:], in0=t1a[:],
                        in1=axt[:, 2 * CW + o2 * BL:2 * CW + (o2 + 1) * BL],
                        op=OP.add)
                    hh = wk.tile([128, BL], F32, tag="a_hh")
                    nc.scalar.activation(hh[:], t2a[:], AF.Tanh)
                    omu = wk.tile([128, BL], F32, tag="a_omu")
                    nc.gpsimd.tensor_scalar(out=omu[:], in0=ur[:, 0:BL],
                                            scalar1=-1.0, scalar2=1.0,
                                            op0=OP.mult, op1=OP.add)
                    ou_n = wk.tile([128, BL], BF16, tag="a_ou")
                    nc.gpsimd.tensor_tensor(out=ou_n[:], in0=omu[:], in1=hA[:],
                                            op=OP.mult)
                    p_n = wk.tile([128, BL], BF16, tag="a_p")
                    nc.vector.tensor_tensor(out=p_n[:], in0=ur[:, 0:BL],
                                            in1=hh[:], op=OP.mult)
                    nc.gpsimd.tensor_tensor(out=hA[:], in0=ou_n[:], in1=p_n[:],
                                            op=OP.add)
                    pA, ouA = p_n, ou_n
                # ---- interleaved phase A ----
                for k in slot_gather.get(slot, ()):
                    gather_tile(k)
                for k in slot_sq.get(slot, ()):
                    sq_tile(k)
                if slot in slot_rsq:
                    rsq_group(*slot_rsq[slot])
                for k in slot_fin.get(slot, ()):
                    finish_tile(k)

            # =========== aux BCE partials ===================
            # bypass-dep on hA orders the Ln table switch after the last tanh
            hdep = wk.tile([128, 1], F32, tag="hdep")
            nc.vector.tensor_copy(hdep[:], hA[:, 0:1])
            s_all2 = pp.tile([128, NTIL], F32, tag="s_all2")
            nc.vector.tensor_scalar(out=s_all2[:], in0=s_all[:],
                                    scalar1=hdep[:, 0:1], scalar2=None,
                                    op0=OP.bypass)
            pq = pp.tile([128, 2 * NTIL], F32, tag="pq")
            nc.scalar.activation(pq[:, 0:NTIL], s_all2[:], AF.Sigmoid)
            nc.scalar.activation(pq[:, NTIL:2 * NTIL], s_all2[:], AF.Sigmoid,
                                 scale=-1.0)
            lpq = pp.tile([128, 2 * NTIL], F32, tag="lpq")
            nc.scalar.activation(lpq[:], pq[:], AF.Ln)
            dl = wk.tile([128, NTIL], F32, tag="dl")
            nc.vector.tensor_tensor(out=dl[:], in0=lpq[:, 0:NTIL],
                                    in1=lpq[:, NTIL:2 * NTIL], op=OP.subtract)
            c1 = wk.tile([128, 1], F32, tag="c1")
            j1 = wk.tile([128, NTIL], F32, tag="j1")
            nc.vector.scalar_tensor_tensor(out=j1[:], in0=dl[:], scalar=1.0,
                                           in1=y_h_s[:], op0=OP.mult, op1=OP.mult,
                                           accum_out=c1[:])
            c0 = wk.tile([128, 1], F32, tag="c0")
            j2 = wk.tile([128, NTIL], F32, tag="j2")
            nc.vector.tensor_scalar(out=j2[:], in0=lpq[:, NTIL:2 * NTIL],
                                    scalar1=1.0, scalar2=0.0,
                                    op0=OP.mult, op1=OP.add, accum_out=c0[:])
            csum = wk.tile([128, 1], F32, tag="csum")
            nc.vector.tensor_tensor(out=csum[:], in0=c0[:], in1=c1[:], op=OP.add)
            aux_ps = psg.tile([1, 1], F32, tag="gram")
            nc.tensor.matmul(aux_ps[:], csum[:], ones_col[:, 0:1],
                             start=True, stop=True)
            aux_sc = wk.tile([1, 1], F32, tag="aux_sc")
            nc.vector.tensor_copy(aux_sc[:], aux_ps[:])

            # =========== pack + AllGather (transposed, bf16) ============
            stage = pp.tile([D, BL + 1], BF16, tag="stage")
            nc.vector.tensor_copy(stage[:, 0:BL], hA[:])
            nc.vector.tensor_copy(stage[0:1, BL:BL + 1], aux_sc[:])
            nc.sync.dma_start(out=ploc[:], in_=stage[:])
            nc.gpsimd.collective_compute(
                "AllGather", OP.bypass,
                replica_groups=[list(range(NCORES))],
                ins=[ploc[:]], outs=[gall[:]],
            )

            # =========== final MLP ============================
            gview = gall[:].rearrange("(c p) x -> p c x", c=NCORES)
            hT_all = pp.tile([D, B], BF16, tag="hT_all")
            nc.sync.dma_start(
                out=hT_all[:].rearrange("p (c x) -> p c x", c=NCORES),
                in_=gview[:, :, 0:BL])
            aux8 = wk.tile([1, NCORES], BF16, tag="aux8")
            nc.sync.dma_start(out=aux8[0:1, :],
                              in_=gview[0:1, :, BL:BL + 1].rearrange(
                                  "p c x -> p (c x)"))
            aux8f = wk.tile([1, NCORES], F32, tag="aux8f")
            nc.vector.tensor_copy(aux8f[:], aux8[:])
            aux_tot = wk.tile([1, 1], F32, tag="aux_tot")
            nc.vector.reduce_sum(out=aux_tot[:], in_=aux8f[:],
                                 axis=mybir.AxisListType.X)

            def dice(z_ps, pdim):
                jm = wk.tile([pdim, B], F32, tag="dice_jm")
                m = wk.tile([pdim, 1], F32, tag="dice_m")
                nc.vector.tensor_scalar(out=jm[:], in0=z_ps, scalar1=1.0 / B,
                                        scalar2=0.0, op0=OP.mult, op1=OP.add,
                                        accum_out=m[:])
                xc = wk.tile([pdim, B], F32, tag="dice_xc")
                nc.vector.tensor_scalar(out=xc[:], in0=z_ps, scalar1=m[:],
                                        scalar2=None, op0=OP.subtract)
                jv = wk.tile([pdim, B], F32, tag="dice_jv")
                vs = wk.tile([pdim, 1], F32, tag="dice_vs")
                nc.vector.scalar_tensor_tensor(out=jv[:], in0=xc[:],
                                               scalar=1.0 / B, in1=xc[:],
                                               op0=OP.mult, op1=OP.mult,
                                               accum_out=vs[:])
                nc.vector.tensor_scalar_add(out=vs[:], in0=vs[:], scalar1=EPS_BN)
                inv = wk.tile([pdim, 1], F32, tag="dice_inv")
                _rsqrt(nc, wk, vs[:], inv[:], [pdim, 1], iters=2)
                pr = wk.tile([pdim, B], F32, tag="dice_p")
                nc.scalar.activation(pr[:], xc[:], AF.Sigmoid, scale=inv[:, 0:1])
                nc.vector.tensor_scalar(out=pr[:], in0=pr[:], scalar1=1 - DICE_A,
                                        scalar2=DICE_A, op0=OP.mult, op1=OP.add)
                zd = wk.tile([pdim, B], BF16, tag="dice_zd")
                nc.vector.tensor_tensor(out=zd[:], in0=z_ps, in1=pr[:], op=OP.mult)
                return zd

            z1_ps = pmlp.tile([128, B], F32, tag="mlp")
            nc.tensor.matmul(z1_ps[:], W1a_s[:], hT_all[:], start=True, stop=False)
            nc.tensor.matmul(z1_ps[:], W1b_s[:], itT[:], start=False, stop=False)
            nc.tensor.matmul(z1_ps[:], b1_s[0:1, :], ones_b[0:1, :],
                             start=False, stop=True)
            z1d = dice(z1_ps[:], 128)

            z2_ps = pmlp.tile([D2, B], F32, tag="mlp")
            nc.tensor.matmul(z2_ps[:], W2_s[:, :], z1d[:], start=True, stop=False)
            nc.tensor.matmul(z2_ps[:], b2_s[0:1, :], ones_b[0:1, :],
                             start=False, stop=True)
            z2d = dice(z2_ps[:], D2)

            s_ps = pmlp.tile([1, B], F32, tag="mlp")
            nc.tensor.matmul(s_ps[:], Wf_s[:, 0:1], z2d[:], start=True, stop=False)
            nc.tensor.matmul(s_ps[:], bf_s[0:1, 0:1], ones_b[0:1, :],
                             start=False, stop=True)

            pq2 = wk.tile([1, 2 * B], F32, tag="pq2")
            nc.scalar.activation(pq2[:, 0:B], s_ps[:], AF.Sigmoid)
            nc.scalar.activation(pq2[:, B:2 * B], s_ps[:], AF.Sigmoid, scale=-1.0)
            lpq2 = wk.tile([1, 2 * B], F32, tag="lpq2")
            nc.scalar.activation(lpq2[:], pq2[:], AF.Ln)
            d2_ = wk.tile([1, B], F32, tag="d2_")
            nc.vector.tensor_tensor(out=d2_[:], in0=lpq2[:, 0:B],
                                    in1=lpq2[:, B:2 * B], op=OP.subtract)
            r1 = wk.tile([1, 1], F32, tag="r1")
            j3 = wk.tile([1, B], F32, tag="j3")
            nc.vector.scalar_tensor_tensor(out=j3[:], in0=d2_[:], scalar=1.0,
                                           in1=y_t_s[:], op0=OP.mult, op1=OP.mult,
                                           accum_out=r1[:])
            r0_ = wk.tile([1, 1], F32, tag="r0_")
            j4 = wk.tile([1, B], F32, tag="j4")
            nc.vector.tensor_scalar(out=j4[:], in0=lpq2[:, B:2 * B],
                                    scalar1=1.0, scalar2=0.0,
                                    op0=OP.mult, op1=OP.add, accum_out=r0_[:])
            rec = wk.tile([1, 1], F32, tag="rec")
            nc.vector.tensor_tensor(out=rec[:], in0=r0_[:], in1=r1[:], op=OP.add)

            nc.vector.tensor_scalar_mul(out=aux_tot[:], in0=aux_tot[:],
                                        scalar1=-ALPHA / (B * L))
            nc.vector.tensor_scalar_mul(out=rec[:], in0=rec[:], scalar1=-1.0 / B)
            res = wk.tile([1, 1], F32, tag="res")
            nc.vector.tensor_tensor(out=res[:], in0=aux_tot[:], in1=rec[:],
                                    op=OP.add)
            nc.sync.dma_start(out=out_p[:], in_=res[:])
    nc.compile()
    return nc


_NC_CACHE = None


def _get_nc():
    global _NC_CACHE
    if _NC_CACHE is None:
        _NC_CACHE = build_bass()
    return _NC_CACHE


def _prep_inputs(inputs):
    import ml_dtypes
    bf16 = ml_dtypes.bfloat16
    f32 = np.float32
    emb = np.ascontiguousarray(inputs["emb"], dtype=f32)
    seqs = np.asarray(inputs["history_seqs"])
    labs = np.asarray(inputs["history_labels"])
    tgt = np.asarray(inputs["target_item"])
    tl = np.asarray(inputs["target_label"]).astype(f32)

    w_ih = np.asarray(inputs["w_ih"], dtype=f32)
    w_hh = np.asarray(inputs["w_hh"], dtype=f32)
    b_ih = np.asarray(inputs["b_ih"], dtype=f32)
    b_hh = np.asarray(inputs["b_hh"], dtype=f32)
    wihT = np.ascontiguousarray(w_ih.T).astype(bf16)
    whhT = np.ascontiguousarray(w_hh.T).astype(bf16)
    b_rz = (b_ih[:2 * D] + b_hh[:2 * D]).reshape(1, 2 * D).astype(bf16)
    b_n = b_ih[2 * D:].reshape(1, D).astype(bf16)
    bhhn = b_hh[2 * D:].reshape(1, D)

    Wall = np.ascontiguousarray(
        np.concatenate([inputs["Wu"], inputs["Wr"], inputs["Wh"]], axis=1),
        dtype=f32).astype(bf16)
    Uall = np.ascontiguousarray(
        np.concatenate([inputs["Uu"], inputs["Ur"], inputs["Uh"]], axis=1),
        dtype=f32).astype(bf16)
    b_urh = np.concatenate(
        [np.asarray(inputs["bu"], dtype=f32).reshape(-1),
         np.asarray(inputs["br"], dtype=f32).reshape(-1),
         np.asarray(inputs["bh"], dtype=f32).reshape(-1)]).reshape(1, 3 * D).astype(bf16)

    W1 = np.ascontiguousarray(inputs["W1"], dtype=f32).astype(bf16)
    b1 = np.asarray(inputs["b1"], dtype=f32).reshape(1, D).astype(bf16)
    W2 = np.ascontiguousarray(inputs["W2"], dtype=f32).astype(bf16)
    b2 = np.asarray(inputs["b2"], dtype=f32).reshape(1, D2).astype(bf16)
    Wf = np.ascontiguousarray(inputs["Wf"], dtype=f32).astype(bf16)
    bf_a = np.asarray(inputs["bf"], dtype=f32).reshape(1, 1).astype(bf16)
    h0 = np.asarray(inputs["h0"], dtype=f32)
    y_t_full = tl.reshape(1, B)
    idx_t_full = np.ascontiguousarray(tgt.reshape(2, 128).T).astype(np.int32)

    shared = dict(emb=emb, wihT=wihT, whhT=whhT, b_rz=b_rz, b_n=b_n, bhhn=bhhn,
                  Wall=Wall, Uall=Uall, b_urh=b_urh, W1=W1, b1=b1, W2=W2, b2=b2,
                  Wf=Wf, bf=bf_a, y_t=y_t_full, idx_t=idx_t_full)
    in_maps = []
    for c in range(NCORES):
        sl = slice(c * BL, (c + 1) * BL)
        idx_f = np.ascontiguousarray(seqs[sl].T).reshape(-1)
        idx_hc = np.ascontiguousarray(
            idx_f.reshape(NTIL, 128).T).astype(np.int32)
        y_f = np.ascontiguousarray(labs[sl, :, 0].T).reshape(-1).astype(f32)
        y_hc = np.ascontiguousarray(y_f.reshape(NTIL, 128).T)
        h0T = np.ascontiguousarray(h0[sl].T).astype(bf16)
        m = dict(shared)
        m.update(idx_h=idx_hc, y_h=y_hc, h0T=h0T)
        in_maps.append(m)
    return in_maps


def kernel(**inputs) -> np.ndarray:
    nc = _get_nc()
    in_maps = _prep_inputs(inputs)
    res = run_bass_kernel_spmd(nc, in_maps, core_ids=list(range(NCORES)))
    out = np.asarray(res.results[0]["out"], dtype=np.float32)
    return out.reshape(())
